# revision 58
# baseline (speedup 1.0000x reference)
"""Trainium2 Bass kernel: batched causal single-head self-attention.

Reference computation (per batch b):
    q = x @ Wq; k = x @ Wk; v = x @ Wv          # [T, H] each, contraction over E
    S = (q @ k^T) / sqrt(H)                     # [T, T]
    P = softmax(causal_mask(S), axis=-1)
    out = P @ v                                 # [T, H]

Shapes: x [512, 256, 384] f32, W* [384, 64] f32, out [512, 256, 64] f32.
Sharding: pure data parallel, 64 batches per NeuronCore across 8 cores.

Device algorithm, quad-granular (4 batches per step), matmuls bf16,
software-pipelined so quad i+1's projections overlap quad i's attention:
  - host ships x^T quad-contiguous ([qd, p, s, c, t]): input DMA = 128
    descriptors x 6KB contiguous.
  - [k^T; q^T] = [Wk|Wq]^T @ xT  (packed 128-wide stationary, 3 E-chunks,
    512-col moving, one PSUM tile per batch-pair).  k^T/q^T staged to
    separate base-0 SBUF tiles (ScalarE/VectorE) so the scores matmul gets
    same-base K=64 operands, as walrus requires.
  - v computed directly in [t, h] layout: stationary = x^T chunk (FWL makes
    the 24 small LDWs cheap), moving = Wv chunk. No PE transposes.
  - v_aug: persistent SBUF tiles with a ones column at h=64 -> the out
    matmul's column 64 is the softmax denominator for free.
  - S^T  = k^T-chunk.T @ q^T     ([tk, tq] layout; lower-left T/4 block skipped)
  - P    = exp(0.125 * S^T)      (ScalarE, per batch-pair; no max-subtraction
    needed, |s| < ~45)
  - P   *= causal 0/1 mask       (diagonal blocks only; VectorE + GpSimdE,
    VectorE-only on the last quad's serial tail)
  - out_aug[tq, 0:65] = P^T V_aug; col 64 = denominator; divided on HOST.
    Output DMA'd in device layout [p, qd, s, j, h], un-scrambled on host.
  - 10 dummy 512-col matmuls at kernel start trip the PE HAM clock gate to
    2.4 GHz while the first input DMAs are still landing.
"""

import numpy as np
import ml_dtypes

B, T, E, H = 512, 256, 384, 64
NCORES = 8
BPC = B // NCORES  # 64
P = 128
EC = E // P  # 3
HP1 = H + 1  # 65

_cache: dict = {}


def _install_ntff_hook():
    """Shim antenv.axon_hooks (absent in this image) so run_bass_kernel_spmd
    trace=True can capture NTFF profiles via the axon .so's C ABI."""
    import contextlib
    import ctypes
    import sys
    import types

    if "antenv.axon_hooks" in sys.modules:
        return
    so_path = "/opt/axon/libaxon_pjrt.so"
    lib = ctypes.CDLL(so_path)
    if not hasattr(lib, "axon_start_nrt_profile"):
        return
    lib.axon_start_nrt_profile.argtypes = [
        ctypes.POINTER(ctypes.c_int64),
        ctypes.c_size_t,
    ]
    lib.axon_start_nrt_profile.restype = ctypes.c_int64
    lib.axon_stop_nrt_profile.argtypes = [ctypes.c_char_p]
    lib.axon_stop_nrt_profile.restype = ctypes.c_int64

    @contextlib.contextmanager
    def _hook(output_dir, device_ids):
        import jax

        jax.devices()
        if device_ids:
            ids = (ctypes.c_int64 * len(device_ids))(*device_ids)
            rc = lib.axon_start_nrt_profile(ids, len(device_ids))
        else:
            rc = lib.axon_start_nrt_profile(None, 0)
        if rc != 0:
            raise RuntimeError(f"axon_start_nrt_profile rc={rc}")
        try:
            yield
        finally:
            n = lib.axon_stop_nrt_profile(str(output_dir).encode())
            if n < 0:
                raise RuntimeError(f"axon_stop_nrt_profile rc={n}")
            print(f"profile: {n} file(s) written to {output_dir}", file=sys.stderr)

    mod = types.ModuleType("antenv.axon_hooks")
    _state = {"hook": _hook}
    mod.get_axon_ntff_profile_hook = lambda: _state["hook"]
    mod.set_axon_ntff_profile_hook = lambda h: _state.__setitem__("hook", h)
    sys.modules["antenv.axon_hooks"] = mod


def _build_program(bpc):
    import concourse.bacc as bacc
    import concourse.mybir as mybir
    import concourse.tile as tile

    f32 = mybir.dt.float32
    bf16 = mybir.dt.bfloat16
    Exp = mybir.ActivationFunctionType.Exp
    Mult = mybir.AluOpType.mult

    nc = bacc.Bacc(
        "TRN2",
        target_bir_lowering=False,
        debug=False,
        enable_asserts=False,
        num_devices=NCORES,
    )
    Q = 4
    assert bpc % Q == 0
    nquads = bpc // Q

    xt_d = nc.dram_tensor("xt", [nquads, P, Q, EC, T], bf16, kind="ExternalInput").ap()
    wkq_d = nc.dram_tensor("wkq", [P, EC, P], bf16, kind="ExternalInput").ap()
    wv_d = nc.dram_tensor("wv", [P, EC, H], bf16, kind="ExternalInput").ap()
    iden_d = nc.dram_tensor("iden", [P, P], bf16, kind="ExternalInput").ap()
    mb_d = nc.dram_tensor("mb", [P, 2, P], bf16, kind="ExternalInput").ap()
    out_d = nc.dram_tensor(
        "out", [P, nquads, Q, 2, HP1], f32, kind="ExternalOutput"
    ).ap()

    with tile.TileContext(nc) as tc:
        with (
            tc.tile_pool(name="const", bufs=1) as constp,
            tc.tile_pool(name="xin", bufs=6) as xpool,
            tc.tile_pool(name="kq", bufs=5) as kqpool,
            tc.tile_pool(name="psb", bufs=4) as ppool,
            tc.tile_pool(name="osb", bufs=3) as opool,
            tc.tile_pool(name="ps_qk", bufs=2, space="PSUM") as ps_qk,
            tc.tile_pool(name="ps_v", bufs=1, space="PSUM") as ps_v,
            tc.tile_pool(name="ps_s", bufs=2, space="PSUM") as ps_s,
            tc.tile_pool(name="ps_o", bufs=1, space="PSUM") as ps_o,
        ):
            # first input quad BEFORE the consts: it's the long pole for the
            # first projection (consts are tiny and queue behind it)
            xt0 = xpool.tile([P, Q, EC, T], bf16, name="xt")
            nc.sync.dma_start(xt0, xt_d[0])
            wkq = constp.tile([P, EC, P], bf16)
            nc.sync.dma_start(wkq, wkq_d)
            wv = constp.tile([P, EC, H], bf16)
            nc.sync.dma_start(wv, wv_d)
            iden = constp.tile([P, P], bf16)
            nc.sync.dma_start(iden, iden_d)
            mb = constp.tile([P, 2, P], bf16)
            nc.sync.dma_start(mb, mb_d)
            # persistent v_aug tiles (manual double-buffer) with ones column
            vaugs = []
            for i in range(2):
                va = constp.tile([P, Q, 2, HP1], bf16, name=f"vaug{i}")
                nc.vector.memset(va[:, :, :, H : H + 1], 1.0)
                vaugs.append(va)

            # HAM warmup: ~3.4us of dummy matmuls (no DMA dependency) so the
            # PE clock gate is at 8/8 by the time real work arrives.
            wrm = constp.tile([P, 2, T], bf16, name="wrm")
            nc.vector.memset(wrm, 0.0)
            wp = ps_s.tile([P, 2, 4, P], f32, name="s_ps")
            for i in range(10):
                nc.tensor.matmul(
                    wp[:, 0, :, :], wrm[:, 0, 0:P], wrm, start=True, stop=True
                )

            def emit_produce(qd):
                """Projection matmuls for quad qd (copies emitted separately
                so they don't block the current quad's exps in the engine
                queues)."""
                if qd == 0:
                    xt = xt0
                else:
                    xt = xpool.tile([P, Q, EC, T], bf16, name="xt")
                    nc.sync.dma_start(xt, xt_d[qd])

                kqs = []
                for pr in range(2):
                    s0 = 2 * pr
                    qk_ps = ps_qk.tile([P, 2, T], f32, name="qk_ps")  # 1 bank
                    for c in range(EC):
                        nc.tensor.matmul(
                            qk_ps,
                            wkq[:, c, :],
                            xt[:, s0 : s0 + 2, c, :],
                            start=(c == 0),
                            stop=(c == EC - 1),
                        )
                    k_sb = kqpool.tile([H, 2, T], bf16, name="k_sb")
                    q_sb = kqpool.tile([H, 2, T], bf16, name="q_sb")
                    nc.scalar.copy(k_sb, qk_ps[0:H])
                    nc.vector.tensor_copy(q_sb, qk_ps[H:P])
                    kqs.append((k_sb, q_sb))
                v_ps = ps_v.tile([P, Q, 2, H], f32, name="v_ps")  # 1 bank
                for s in range(Q):
                    for j in range(2):
                        for c in range(EC):
                            nc.tensor.matmul(
                                v_ps[:, s, j, :],
                                xt[:, s, c, j * P : (j + 1) * P],
                                wv[:, c, :],
                                start=(c == 0),
                                stop=(c == EC - 1),
                            )

                v_aug = vaugs[qd % 2]
                nc.vector.tensor_copy(v_aug[:, :, :, 0:H], v_ps)
                return kqs, v_aug

            def emit_consume(qd, kqs, v_aug, last=False):
                """Scores, softmax, mask, out for quad qd."""
                p_sb = ppool.tile([P, Q, 3, P], bf16, name="p_sb")
                for pr in range(2):
                    s0 = 2 * pr
                    k_sb, q_sb = kqs[pr]
                    s_ps = ps_s.tile([P, 2, 4, P], f32, name="s_ps")  # 2 banks
                    for si in range(2):
                        # causal mask as a PSUM pre-write: -240 on the masked
                        # half of the diagonal blocks (exp -> ~1e-13), zeros
                        # on the full block; scores accumulate on top, so the
                        # exp output needs no separate mask pass
                        nc.tensor.matmul(
                            s_ps[:, si, 0:2, :],
                            iden,
                            mb,
                            start=True,
                            stop=False,
                        )
                        nc.tensor.matmul(
                            s_ps[:, si, 0:2, :],
                            k_sb[:, si, 0:P],
                            q_sb[:, si, :],
                            start=False,
                            stop=True,
                        )
                        nc.tensor.matmul(
                            s_ps[:, si, 2, :],
                            iden,
                            mb[:, 0, :],
                            start=True,
                            stop=False,
                        )
                        nc.tensor.matmul(
                            s_ps[:, si, 2, :],
                            k_sb[:, si, P:T],
                            q_sb[:, si, P:T],
                            start=False,
                            stop=True,
                        )
                    nc.scalar.activation(
                        p_sb[:, s0 : s0 + 2, :, :],
                        s_ps[:, :, 0:3, :],
                        Exp,
                        scale=0.125,
                    )

                o_sb = opool.tile([P, Q, 2, HP1], f32, name="o_sb")
                for pr in range(2):
                    s0 = 2 * pr
                    o_ps = ps_o.tile([P, 2, 2, HP1], f32, name="o_ps")  # 1 bank
                    for si in range(2):
                        s = s0 + si
                        nc.tensor.matmul(
                            o_ps[:, si, 0, :],
                            p_sb[:, s, 0, :],
                            v_aug[:, s, 0, :],
                            start=True,
                            stop=True,
                        )
                        nc.tensor.matmul(
                            o_ps[:, si, 1, :],
                            p_sb[:, s, 1, :],
                            v_aug[:, s, 0, :],
                            start=True,
                            stop=False,
                        )
                        nc.tensor.matmul(
                            o_ps[:, si, 1, :],
                            p_sb[:, s, 2, :],
                            v_aug[:, s, 1, :],
                            start=False,
                            stop=True,
                        )
                    nc.vector.tensor_copy(o_sb[:, s0 : s0 + 2, :, :], o_ps)

                nc.sync.dma_start(out_d[:, qd], o_sb)

            # software pipeline: produce runs one quad ahead of consume
            staged = emit_produce(0)
            for qd in range(nquads):
                nxt = emit_produce(qd + 1) if qd + 1 < nquads else None
                emit_consume(qd, *staged, last=(qd + 1 == nquads))
                staged = nxt

    nc.compile()
    return nc


def _prep_inputs(x, Wq, Wk, Wv, bpc):
    bf = ml_dtypes.bfloat16
    nb = NCORES * bpc
    nq = bpc // 4
    x = np.asarray(x, dtype=np.float32)[:nb]
    # [b, t, e] -> per core [qd, p, s, c, t] with b = qd*4+s, e = c*128+p
    xt = np.ascontiguousarray(
        x.reshape(NCORES, nq, 4, T, EC, P).transpose(0, 1, 5, 2, 4, 3)
    ).astype(bf)
    wkq = np.concatenate(
        [np.asarray(Wk, np.float32), np.asarray(Wq, np.float32)], axis=1
    )  # [E, 128]: k^T on PSUM partitions 0:64, q^T on 64:128
    wkq = np.ascontiguousarray(wkq.reshape(EC, P, P).transpose(1, 0, 2)).astype(bf)
    wv = np.ascontiguousarray(
        np.asarray(Wv, np.float32).reshape(EC, P, H).transpose(1, 0, 2)
    ).astype(bf)
    iden = np.eye(P, dtype=np.float32).astype(bf)
    mbias = np.where(
        np.arange(P)[:, None] > np.arange(P)[None, :], -240.0, 0.0
    ).astype(np.float32)
    mb = np.stack([mbias, np.zeros((P, P), np.float32)], axis=1).astype(bf)
    per_core = []
    for c in range(NCORES):
        per_core.append(
            {
                "xt": xt[c],
                "wkq": wkq,
                "wv": wv,
                "iden": iden,
                "mb": mb,
            }
        )
    return per_core


def kernel(x, Wq, Wk, Wv, _trace=False, _bpc=BPC):
    """Full inputs in, full output out. Shards batch dim over 8 NeuronCores."""
    from concourse import bass_utils

    if _trace:
        _install_ntff_hook()

    key = ("prog", _bpc)
    if key not in _cache:
        _cache[key] = _build_program(_bpc)
    nc = _cache[key]

    in_maps = _prep_inputs(x, Wq, Wk, Wv, _bpc)
    res = bass_utils.run_bass_kernel_spmd(
        nc, in_maps, core_ids=list(range(NCORES)), trace=_trace
    )
    _cache["last_result"] = res
    nq = _bpc // 4
    # device layout [p, qd, s, j, h] -> [b, t, h] with b=qd*4+s, t=j*128+p;
    # col 64 is the softmax denominator -> divide here
    outs = []
    for r in res.results:
        o = r["out"].reshape(P, nq, 4, 2, HP1).transpose(1, 2, 3, 0, 4)
        o = np.ascontiguousarray(o).reshape(_bpc, T, HP1)
        outs.append(o[:, :, 0:H] / o[:, :, H : H + 1])
    out = np.concatenate(outs, axis=0)
    return out.astype(np.float32)


# revision 59
# speedup vs baseline: 1.3758x; 1.3758x over previous
"""Trainium2 Bass kernel: batched causal single-head self-attention.

Reference computation (per batch b):
    q = x @ Wq; k = x @ Wk; v = x @ Wv          # [T, H] each, contraction over E
    S = (q @ k^T) / sqrt(H)                     # [T, T]
    P = softmax(causal_mask(S), axis=-1)
    out = P @ v                                 # [T, H]

Shapes: x [512, 256, 384] f32, W* [384, 64] f32, out [512, 256, 64] f32.
Sharding: pure data parallel, 64 batches per NeuronCore across 8 cores.

Device algorithm, quad-granular (4 batches per step), matmuls bf16,
software-pipelined so quad i+1's projections overlap quad i's attention:
  - host ships x^T quad-contiguous ([qd, p, s, c, t]): input DMA = 128
    descriptors x 6KB contiguous.
  - [k^T; q^T] = [Wk|Wq]^T @ xT  (packed 128-wide stationary, 3 E-chunks,
    512-col moving, one PSUM tile per batch-pair).  k^T/q^T staged to
    separate base-0 SBUF tiles (ScalarE/VectorE) so the scores matmul gets
    same-base K=64 operands, as walrus requires.
  - v computed directly in [t, h] layout: stationary = x^T chunk (FWL makes
    the 24 small LDWs cheap), moving = Wv chunk. No PE transposes.
  - v_aug: persistent SBUF tiles with a ones column at h=64 -> the out
    matmul's column 64 is the softmax denominator for free.
  - S^T  = k^T-chunk.T @ q^T     ([tk, tq] layout; lower-left T/4 block skipped)
  - P    = exp(0.125 * S^T)      (ScalarE, per batch-pair; no max-subtraction
    needed, |s| < ~45)
  - P   *= causal 0/1 mask       (diagonal blocks only; VectorE + GpSimdE,
    VectorE-only on the last quad's serial tail)
  - out_aug[tq, 0:65] = P^T V_aug; col 64 = denominator; divided on HOST.
    Output DMA'd in device layout [p, qd, s, j, h], un-scrambled on host.
  - 10 dummy 512-col matmuls at kernel start trip the PE HAM clock gate to
    2.4 GHz while the first input DMAs are still landing.
"""

import numpy as np
import ml_dtypes

B, T, E, H = 512, 256, 384, 64
NCORES = 8
BPC = B // NCORES  # 64
P = 128
EC = E // P  # 3
HP1 = H + 1  # 65

_cache: dict = {}


def _install_ntff_hook():
    """Shim antenv.axon_hooks (absent in this image) so run_bass_kernel_spmd
    trace=True can capture NTFF profiles via the axon .so's C ABI."""
    import contextlib
    import ctypes
    import sys
    import types

    if "antenv.axon_hooks" in sys.modules:
        return
    so_path = "/opt/axon/libaxon_pjrt.so"
    lib = ctypes.CDLL(so_path)
    if not hasattr(lib, "axon_start_nrt_profile"):
        return
    lib.axon_start_nrt_profile.argtypes = [
        ctypes.POINTER(ctypes.c_int64),
        ctypes.c_size_t,
    ]
    lib.axon_start_nrt_profile.restype = ctypes.c_int64
    lib.axon_stop_nrt_profile.argtypes = [ctypes.c_char_p]
    lib.axon_stop_nrt_profile.restype = ctypes.c_int64

    @contextlib.contextmanager
    def _hook(output_dir, device_ids):
        import jax

        jax.devices()
        if device_ids:
            ids = (ctypes.c_int64 * len(device_ids))(*device_ids)
            rc = lib.axon_start_nrt_profile(ids, len(device_ids))
        else:
            rc = lib.axon_start_nrt_profile(None, 0)
        if rc != 0:
            raise RuntimeError(f"axon_start_nrt_profile rc={rc}")
        try:
            yield
        finally:
            n = lib.axon_stop_nrt_profile(str(output_dir).encode())
            if n < 0:
                raise RuntimeError(f"axon_stop_nrt_profile rc={n}")
            print(f"profile: {n} file(s) written to {output_dir}", file=sys.stderr)

    mod = types.ModuleType("antenv.axon_hooks")
    _state = {"hook": _hook}
    mod.get_axon_ntff_profile_hook = lambda: _state["hook"]
    mod.set_axon_ntff_profile_hook = lambda h: _state.__setitem__("hook", h)
    sys.modules["antenv.axon_hooks"] = mod


def _build_program(bpc):
    import concourse.bacc as bacc
    import concourse.mybir as mybir
    import concourse.tile as tile

    f32 = mybir.dt.float32
    bf16 = mybir.dt.bfloat16
    Exp = mybir.ActivationFunctionType.Exp
    Mult = mybir.AluOpType.mult

    nc = bacc.Bacc(
        "TRN2",
        target_bir_lowering=False,
        debug=False,
        enable_asserts=False,
        num_devices=NCORES,
    )
    Q = 4
    assert bpc % Q == 0
    nquads = bpc // Q

    xt_d = nc.dram_tensor("xt", [nquads, P, Q, EC, T], bf16, kind="ExternalInput").ap()
    wkq_d = nc.dram_tensor("wkq", [P, EC, P], bf16, kind="ExternalInput").ap()
    wv_d = nc.dram_tensor("wv", [P, EC, H], bf16, kind="ExternalInput").ap()
    um_d = nc.dram_tensor("um", [P, P], bf16, kind="ExternalInput").ap()
    out_d = nc.dram_tensor(
        "out", [P, nquads, Q, 2, HP1], f32, kind="ExternalOutput"
    ).ap()

    with tile.TileContext(nc) as tc:
        with (
            tc.tile_pool(name="const", bufs=1) as constp,
            tc.tile_pool(name="xin", bufs=6) as xpool,
            tc.tile_pool(name="kq", bufs=5) as kqpool,
            tc.tile_pool(name="psb", bufs=4) as ppool,
            tc.tile_pool(name="osb", bufs=3) as opool,
            tc.tile_pool(name="ps_qk", bufs=2, space="PSUM") as ps_qk,
            tc.tile_pool(name="ps_v", bufs=1, space="PSUM") as ps_v,
            tc.tile_pool(name="ps_s", bufs=2, space="PSUM") as ps_s,
            tc.tile_pool(name="ps_o", bufs=1, space="PSUM") as ps_o,
        ):
            # first input quad BEFORE the consts: it's the long pole for the
            # first projection (consts are tiny and queue behind it)
            xt0 = xpool.tile([P, Q, EC, T], bf16, name="xt")
            nc.sync.dma_start(xt0, xt_d[0])
            wkq = constp.tile([P, EC, P], bf16)
            nc.sync.dma_start(wkq, wkq_d)
            wv = constp.tile([P, EC, H], bf16)
            nc.sync.dma_start(wv, wv_d)
            um = constp.tile([P, P], bf16)
            nc.sync.dma_start(um, um_d)
            # persistent v_aug tiles (manual double-buffer) with ones column
            vaugs = []
            for i in range(2):
                va = constp.tile([P, Q, 2, HP1], bf16, name=f"vaug{i}")
                nc.vector.memset(va[:, :, :, H : H + 1], 1.0)
                vaugs.append(va)

            # HAM warmup: ~3.4us of dummy matmuls (no DMA dependency) so the
            # PE clock gate is at 8/8 by the time real work arrives.
            wrm = constp.tile([P, 2, T], bf16, name="wrm")
            nc.vector.memset(wrm, 0.0)
            wp = ps_s.tile([P, 2, 4, P], f32, name="s_ps")
            for i in range(10):
                nc.tensor.matmul(
                    wp[:, 0, :, :], wrm[:, 0, 0:P], wrm, start=True, stop=True
                )

            def emit_produce(qd):
                """Projection matmuls for quad qd (copies emitted separately
                so they don't block the current quad's exps in the engine
                queues)."""
                if qd == 0:
                    xt = xt0
                else:
                    xt = xpool.tile([P, Q, EC, T], bf16, name="xt")
                    nc.sync.dma_start(xt, xt_d[qd])

                kqs = []
                for pr in range(2):
                    s0 = 2 * pr
                    qk_ps = ps_qk.tile([P, 2, T], f32, name="qk_ps")  # 1 bank
                    for c in range(EC):
                        nc.tensor.matmul(
                            qk_ps,
                            wkq[:, c, :],
                            xt[:, s0 : s0 + 2, c, :],
                            start=(c == 0),
                            stop=(c == EC - 1),
                        )
                    k_sb = kqpool.tile([H, 2, T], bf16, name="k_sb")
                    q_sb = kqpool.tile([H, 2, T], bf16, name="q_sb")
                    nc.scalar.copy(k_sb, qk_ps[0:H])
                    nc.vector.tensor_copy(q_sb, qk_ps[H:P])
                    kqs.append((k_sb, q_sb))
                v_ps = ps_v.tile([P, Q, 2, H], f32, name="v_ps")  # 1 bank
                for s in range(Q):
                    for j in range(2):
                        for c in range(EC):
                            nc.tensor.matmul(
                                v_ps[:, s, j, :],
                                xt[:, s, c, j * P : (j + 1) * P],
                                wv[:, c, :],
                                start=(c == 0),
                                stop=(c == EC - 1),
                            )

                v_aug = vaugs[qd % 2]
                nc.vector.tensor_copy(v_aug[:, :, :, 0:H], v_ps)
                return kqs, v_aug

            def emit_consume(qd, kqs, v_aug, last=False):
                """Scores, softmax, mask, out for quad qd."""
                p_sb = ppool.tile([P, Q, 3, P], bf16, name="p_sb")
                for pr in range(2):
                    s0 = 2 * pr
                    k_sb, q_sb = kqs[pr]
                    s_ps = ps_s.tile([P, 2, 4, P], f32, name="s_ps")  # 2 banks
                    for si in range(2):
                        nc.tensor.matmul(
                            s_ps[:, si, 0:2, :],
                            k_sb[:, si, 0:P],
                            q_sb[:, si, :],
                            start=True,
                            stop=True,
                        )
                        nc.tensor.matmul(
                            s_ps[:, si, 2, :],
                            k_sb[:, si, P:T],
                            q_sb[:, si, P:T],
                            start=True,
                            stop=True,
                        )
                    nc.scalar.activation(
                        p_sb[:, s0 : s0 + 2, :, :],
                        s_ps[:, :, 0:3, :],
                        Exp,
                        scale=0.125,
                    )

                # causal mask on the diagonal blocks (SBUF-only)
                nc.vector.tensor_tensor(
                    p_sb[:, :, 0, :],
                    p_sb[:, :, 0, :],
                    um[:, None, :].to_broadcast([P, Q, P]),
                    Mult,
                )
                # GpSimd is slow per-op; for the last quad the mask is on the
                # serial tail, so keep it on the faster VectorE there
                eng2 = nc.vector if last else nc.gpsimd
                eng2.tensor_tensor(
                    p_sb[:, :, 2, :],
                    p_sb[:, :, 2, :],
                    um[:, None, :].to_broadcast([P, Q, P]),
                    Mult,
                )

                o_sb = opool.tile([P, Q, 2, HP1], f32, name="o_sb")
                for pr in range(2):
                    s0 = 2 * pr
                    o_ps = ps_o.tile([P, 2, 2, HP1], f32, name="o_ps")  # 1 bank
                    for si in range(2):
                        s = s0 + si
                        nc.tensor.matmul(
                            o_ps[:, si, 0, :],
                            p_sb[:, s, 0, :],
                            v_aug[:, s, 0, :],
                            start=True,
                            stop=True,
                        )
                        nc.tensor.matmul(
                            o_ps[:, si, 1, :],
                            p_sb[:, s, 1, :],
                            v_aug[:, s, 0, :],
                            start=True,
                            stop=False,
                        )
                        nc.tensor.matmul(
                            o_ps[:, si, 1, :],
                            p_sb[:, s, 2, :],
                            v_aug[:, s, 1, :],
                            start=False,
                            stop=True,
                        )
                    nc.vector.tensor_copy(o_sb[:, s0 : s0 + 2, :, :], o_ps)

                nc.sync.dma_start(out_d[:, qd], o_sb)

            # software pipeline: produce runs one quad ahead of consume
            staged = emit_produce(0)
            for qd in range(nquads):
                nxt = emit_produce(qd + 1) if qd + 1 < nquads else None
                emit_consume(qd, *staged, last=(qd + 1 == nquads))
                staged = nxt

    nc.compile()
    return nc


def _prep_inputs(x, Wq, Wk, Wv, bpc):
    bf = ml_dtypes.bfloat16
    nb = NCORES * bpc
    nq = bpc // 4
    x = np.asarray(x, dtype=np.float32)[:nb]
    # [b, t, e] -> per core [qd, p, s, c, t] with b = qd*4+s, e = c*128+p
    xt = np.ascontiguousarray(
        x.reshape(NCORES, nq, 4, T, EC, P).transpose(0, 1, 5, 2, 4, 3)
    ).astype(bf)
    wkq = np.concatenate(
        [np.asarray(Wk, np.float32), np.asarray(Wq, np.float32)], axis=1
    )  # [E, 128]: k^T on PSUM partitions 0:64, q^T on 64:128
    wkq = np.ascontiguousarray(wkq.reshape(EC, P, P).transpose(1, 0, 2)).astype(bf)
    wv = np.ascontiguousarray(
        np.asarray(Wv, np.float32).reshape(EC, P, H).transpose(1, 0, 2)
    ).astype(bf)
    tril01 = (np.arange(P)[:, None] <= np.arange(P)[None, :]).astype(np.float32)
    um = tril01.astype(bf)
    per_core = []
    for c in range(NCORES):
        per_core.append(
            {
                "xt": xt[c],
                "wkq": wkq,
                "wv": wv,
                "um": um,
            }
        )
    return per_core


def kernel(x, Wq, Wk, Wv, _trace=False, _bpc=BPC):
    """Full inputs in, full output out. Shards batch dim over 8 NeuronCores."""
    from concourse import bass_utils

    if _trace:
        _install_ntff_hook()

    key = ("prog", _bpc)
    if key not in _cache:
        _cache[key] = _build_program(_bpc)
    nc = _cache[key]

    in_maps = _prep_inputs(x, Wq, Wk, Wv, _bpc)
    res = bass_utils.run_bass_kernel_spmd(
        nc, in_maps, core_ids=list(range(NCORES)), trace=_trace
    )
    _cache["last_result"] = res
    nq = _bpc // 4
    # device layout [p, qd, s, j, h] -> [b, t, h] with b=qd*4+s, t=j*128+p;
    # col 64 is the softmax denominator -> divide here
    outs = []
    for r in res.results:
        o = r["out"].reshape(P, nq, 4, 2, HP1).transpose(1, 2, 3, 0, 4)
        o = np.ascontiguousarray(o).reshape(_bpc, T, HP1)
        outs.append(o[:, :, 0:H] / o[:, :, H : H + 1])
    out = np.concatenate(outs, axis=0)
    return out.astype(np.float32)
